# revision 21
# baseline (speedup 1.0000x reference)
"""GCN mean-aggregation (DGL copy_src -> mean by dst) on 8 NeuronCores.

Strategy (node-sharded, no collectives, host-packed edge records):
  - Host: nodes are assigned to cores by degree-balanced snake, then
    bin-packed per core into buckets of <=32 nodes and <=256 edges
    (degree-paired two-pointer fill, ~96% tile occupancy).  Each edge
    contributes a 128-byte feature record (64 x bf16 of its src row) and
    a 2-byte dst-slot value, laid out as SBUF images
    [128 partitions = edge%128, nt*128B] / [128, nt*2B], so the device
    streams everything with large sequential HWDGE DMAs (no per-edge
    gather descriptors, no SWDGE).
  - Device (identical program per core):
      * load the dst-slot table once; DVE builds fp8 one-hot tiles
        [128, 32] via batched is_equal(iota, slot) 16 tiles at a time
      * stream the feature table in chunks (quad buffered)
      * per 128-edge tile: one matmul psum[32q:32q+32, :64] +=
        onehot_fp8^T @ feat_bf16 (PE quadrant q = bucket%4, fp8 weights
        + bf16 moving; accumulate over the bucket's tiles)
      * per 4-bucket group: ACT copies psum -> bf16 out image scaled by
        recip = 1/max(deg,1) (host-precomputed, per-partition scalar)
      * 8 segment DMAs write the [128, ngroups*64] bf16 output image
  - Host: un-image per-node rows into the [100000, 64] f32 output.
"""

import bisect
import sys
from contextlib import ExitStack

import numpy as np

sys.path.insert(0, "/opt/trn_rl_repo")

import concourse.bass as bass  # noqa: E402
import concourse.mybir as mybir  # noqa: E402
import concourse.tile as tile  # noqa: E402
from concourse import bacc  # noqa: E402
from concourse.bass_utils import run_bass_kernel_spmd  # noqa: E402

N_NODES = 100000
N_EDGES = 1000000
D_FEAT = 64
N_CORES = 8
SLOTS = 32  # nodes per bucket (one-hot width)
EDGE_CAP = 256  # max edges per bucket
REC = 128  # bytes/record: 64 x bf16 feats
CHUNK = 32  # record tiles per DMA chunk
EDGE_CHUNK = 16  # smaller chunks at stream head/tail
OH_BATCH = 16  # one-hot tiles built per DVE instruction
OUT_SEGS = 12


def _f32_to_bf16_u16(x):
    u = np.ascontiguousarray(x, dtype=np.float32).view(np.uint32)
    r = ((u >> 16) & 1) + 0x7FFF  # round to nearest even
    return ((u + r) >> 16).astype(np.uint16)


def _pack_core(nodes, deg):
    """Exact-fill two-pointer bin-pack with node splitting.

    nodes: deg-desc order.  Returns list of bins; each bin is
    (edge_count, [(node, edges_assigned), ...]) with edge_count == EDGE_CAP
    for all but possibly the last bin, and <= SLOTS members per bin.
    Zero-degree nodes get no slot.
    """
    nodes = [n for n in nodes if deg[n] > 0]
    rem = {int(n): int(deg[n]) for n in nodes}
    bins = []
    i, j = 0, len(nodes) - 1
    cur, s = [], 0

    def close():
        nonlocal cur, s
        if cur:
            bins.append((s, cur))
            cur, s = [], 0

    while i <= j:
        # place the biggest remaining node, splitting across bins as needed
        n0 = int(nodes[i])
        i += 1
        r = rem[n0]
        while r > 0:
            space = EDGE_CAP - s
            if space == 0 or len(cur) >= SLOTS:
                close()
                space = EDGE_CAP
            take = min(r, space)
            cur.append((n0, take))
            s += take
            r -= take
        # fill with small nodes from the tail
        while j >= i and len(cur) < SLOTS and s < EDGE_CAP:
            nj = int(nodes[j])
            d = rem[nj]
            if s + d <= EDGE_CAP:
                cur.append((nj, d))
                s += d
                j -= 1
            else:
                take = EDGE_CAP - s
                cur.append((nj, take))
                rem[nj] = d - take
                s = EDGE_CAP
        if s >= EDGE_CAP or len(cur) >= SLOTS:
            close()
    close()
    return bins


def _prep(embeddings, src, dst):
    src = np.asarray(src, dtype=np.int64)
    dst = np.asarray(dst, dtype=np.int64)

    deg = np.bincount(dst, minlength=N_NODES)
    order = np.argsort(-deg, kind="stable")
    pos = np.arange(N_NODES) % (2 * N_CORES)
    core_pat = np.where(pos < N_CORES, pos, 2 * N_CORES - 1 - pos)
    core_of = np.empty(N_NODES, np.int64)
    core_of[order] = core_pat

    bins_c = []
    for c in range(N_CORES):
        nodes_c = order[core_of[order] == c]
        b = _pack_core(nodes_c, deg)
        b.sort(key=lambda t: -t[0])
        bins_c.append(b)
    nbmax = max(len(b) for b in bins_c)
    ngroups = -(-nbmax // 4)
    nb = ngroups * 4

    tpb = np.zeros(nb, np.int64)
    for b in bins_c:
        for r, (s, _) in enumerate(b):
            tpb[r] = max(tpb[r], -(-s // 128))
    tpb = np.maximum(tpb, 1)
    tile0 = np.zeros(nb + 1, np.int64)
    np.cumsum(tpb, out=tile0[1:])
    nt = int(tile0[-1])

    # per-node slot segments (a split node occupies slots in several buckets;
    # its edges fill them in segment order and the host sums the partial rows)
    recip = (1.0 / np.maximum(deg, 1)).astype(np.float32)
    rt = np.ones((N_CORES, 128, ngroups), np.float32)
    seg_of = [[] for _ in range(N_NODES)]  # node -> [(core, rank, slot, cnt)]
    for c in range(N_CORES):
        for r, (_, members) in enumerate(bins_c[c]):
            g, q = r // 4, r % 4
            for s, (n, cnt) in enumerate(members):
                seg_of[n].append((c, r, s, cnt))
                rt[c, q * 32 + s, g] = recip[n]

    # per-edge segment assignment: edges grouped by dst; the i-th edge of a
    # node goes to the segment covering index i
    eorder0 = np.argsort(dst, kind="stable")
    dcnt = np.bincount(dst, minlength=N_NODES)
    dstart = np.zeros(N_NODES + 1, np.int64)
    np.cumsum(dcnt, out=dstart[1:])
    r_in_node = np.arange(dst.shape[0], dtype=np.int64) - dstart[dst[eorder0]]

    ec = np.empty(dst.shape[0], np.int64)
    erank = np.empty(dst.shape[0], np.int64)
    eslot = np.empty(dst.shape[0], np.int64)
    # fast path: nodes with a single segment
    nseg = np.array([len(s) for s in seg_of], np.int64)
    s0 = np.zeros((N_NODES, 3), np.int64)
    for n in range(N_NODES):
        if seg_of[n]:
            s0[n] = seg_of[n][0][:3]
    dn = dst[eorder0]
    single = nseg[dn] == 1
    ec[single] = s0[dn[single], 0]
    erank[single] = s0[dn[single], 1]
    eslot[single] = s0[dn[single], 2]
    multi_nodes = np.nonzero(nseg > 1)[0]
    for n in multi_nodes:
        lo = dstart[n]
        hi = dstart[n + 1]
        idx = eorder0[lo:hi]  # this node's edges in placement order
        off = 0
        for (c, r, s, cnt) in seg_of[n]:
            sel = slice(lo, hi)
            pick = np.arange(lo, hi)[(r_in_node[lo:hi] >= off) & (r_in_node[lo:hi] < off + cnt)]
            ec[pick] = c
            erank[pick] = r
            eslot[pick] = s
            off += cnt

    # order edges by (core, rank) for bucket-sequential slots
    key = ec * nb + erank
    eorder1 = np.argsort(key, kind="stable")
    ks = key[eorder1]
    kcnt = np.bincount(ks, minlength=N_CORES * nb)
    kstart = np.zeros(kcnt.shape[0] + 1, np.int64)
    np.cumsum(kcnt, out=kstart[1:])
    k_in_bucket = np.arange(ks.shape[0], dtype=np.int64) - kstart[ks]

    et = tile0[erank[eorder1]] + (k_in_bucket >> 7)
    ep = k_in_bucket & 127
    ecc = ec[eorder1]
    esl = eslot[eorder1]
    esrc = src[eorder0][eorder1]

    featb = _f32_to_bf16_u16(embeddings)  # [N, 64] uint16

    FEAT = np.zeros((N_CORES * nt * 128, D_FEAT), np.uint16)
    DSTV = np.full((N_CORES * nt * 128,), SLOTS, np.uint8)
    rows = (ecc * nt + et) * 128 + ep
    FEAT[rows, :] = featb[esrc]
    DSTV[rows] = esl.astype(np.uint8)

    img = np.ascontiguousarray(
        FEAT.reshape(N_CORES, nt, 128, D_FEAT)
        .transpose(0, 2, 1, 3)
        .reshape(N_CORES, 128, nt * D_FEAT)
    ).view(np.int8)  # [C, 128, nt*128B]
    dimg = np.ascontiguousarray(
        DSTV.reshape(N_CORES, nt, 128).transpose(0, 2, 1)
    ).view(np.int8)  # [C, 128, nt] int8

    # unpack plan: per segment-position, (nodes, core, partition, group)
    maxseg = int(nseg.max()) if nseg.size else 0
    unpack = []
    for si in range(maxseg):
        nn = np.nonzero(nseg > si)[0]
        cc_ = np.array([seg_of[n][si][0] for n in nn], np.int64)
        rr_ = np.array([seg_of[n][si][1] for n in nn], np.int64)
        ss_ = np.array([seg_of[n][si][2] for n in nn], np.int64)
        unpack.append((nn, cc_, (rr_ % 4) * 32 + ss_, rr_ // 4))

    return img, dimg, rt, tuple(int(t) for t in tpb), ngroups, unpack


def _build(tpb, ngroups):
    f32 = mybir.dt.float32
    i8 = mybir.dt.int8
    bf16 = mybir.dt.bfloat16
    fp8 = mybir.dt.float8e4
    i32 = mybir.dt.int32
    nb = len(tpb)
    nt = sum(tpb)

    nc = bacc.Bacc("TRN2", target_bir_lowering=False, debug=False)
    tab = nc.dram_tensor("tab", [128, nt * REC], i8, kind="ExternalInput")
    dstv = nc.dram_tensor("dstv", [128, nt], i8, kind="ExternalInput")
    recip = nc.dram_tensor("recip", [128, ngroups], f32, kind="ExternalInput")
    out = nc.dram_tensor(
        "out", [128, ngroups * D_FEAT], bf16, kind="ExternalOutput"
    )

    bounds = [0]
    while bounds[-1] < nt:
        left = nt - bounds[-1]
        sz = (
            EDGE_CHUNK
            if (len(bounds) <= 2 or left <= 2 * EDGE_CHUNK + CHUNK)
            else CHUNK
        )
        bounds.append(min(bounds[-1] + sz, nt))

    with tile.TileContext(nc) as tc, ExitStack() as ctx:
        const_p = ctx.enter_context(tc.tile_pool(name="const", bufs=1))
        tab_p = ctx.enter_context(tc.tile_pool(name="tab", bufs=10))
        oh_p = ctx.enter_context(tc.tile_pool(name="oh", bufs=8))
        ps_p = ctx.enter_context(tc.tile_pool(name="ps", bufs=8, space="PSUM"))
        out_p = ctx.enter_context(tc.tile_pool(name="outp", bufs=1))

        dv = const_p.tile([128, nt], i8)
        rc = const_p.tile([128, ngroups], f32)

        iota_i = const_p.tile([128, SLOTS], i32)
        nc.gpsimd.iota(iota_i[:], pattern=[[1, SLOTS]], base=0, channel_multiplier=0)
        iota_b = const_p.tile([128, SLOTS], i8)
        nc.vector.tensor_copy(out=iota_b[:], in_=iota_i[:])

        oimg = out_p.tile([128, ngroups * D_FEAT], bf16)

        chunks = []

        def chunk_for(t):
            c = bisect.bisect_right(bounds, t) - 1
            while len(chunks) <= c:
                cc = len(chunks)
                t0b, t1b = bounds[cc], bounds[cc + 1]
                ctile = tab_p.tile([128, (t1b - t0b) * REC], i8, tag="chunk")
                nc.sync.dma_start(
                    out=ctile[:], in_=tab[:, t0b * REC : t1b * REC]
                )
                chunks.append(ctile)
            return chunks[c], (t - bounds[c]) * REC

        ohtiles = []

        def oh_for(t):
            c = t // OH_BATCH
            while len(ohtiles) <= c:
                cc = len(ohtiles)
                k = min(OH_BATCH, nt - cc * OH_BATCH)
                oht = oh_p.tile([128, k * SLOTS], fp8, tag="oh")
                nc.vector.tensor_tensor(
                    out=oht[:].rearrange("p (b f) -> p b f", b=k),
                    in0=iota_b[:, None, :].broadcast_to([128, k, SLOTS]),
                    in1=dv[:, cc * OH_BATCH : cc * OH_BATCH + k][
                        :, :, None
                    ].broadcast_to([128, k, SLOTS]),
                    op=mybir.AluOpType.is_equal,
                )
                ohtiles.append(oht)
            return ohtiles[c], (t - c * OH_BATCH) * SLOTS

        seg_end = [((s + 1) * ngroups) // OUT_SEGS for s in range(OUT_SEGS)]

        nc.sync.dma_start(out=dv[:], in_=dstv[:, :])
        chunk_for(0)
        if bounds[1] < nt:
            chunk_for(bounds[1])
        nc.sync.dma_start(out=rc[:], in_=recip[:, :])

        t = 0
        for g in range(ngroups):
            psum = ps_p.tile([128, D_FEAT], f32)
            for q in range(4):
                r = g * 4 + q
                for j in range(tpb[r]):
                    ctile, o = chunk_for(t)
                    oht, oo = oh_for(t)
                    nc.tensor.matmul(
                        out=psum[32 * q : 32 * (q + 1), :],
                        lhsT=oht[:, oo : oo + SLOTS],
                        rhs=ctile[:, o : o + REC].bitcast(bf16),
                        start=(j == 0),
                        stop=(j == tpb[r] - 1),
                        tile_position=(0, 32 * q),
                    )
                    t += 1
            nc.scalar.activation(
                out=oimg[:, g * D_FEAT : (g + 1) * D_FEAT],
                in_=psum[:],
                func=mybir.ActivationFunctionType.Copy,
                scale=rc[:, g : g + 1],
            )
            if g + 1 in seg_end:
                s0 = seg_end.index(g + 1)
                lo = 0 if s0 == 0 else seg_end[s0 - 1]
                nc.sync.dma_start(
                    out=out[:, lo * D_FEAT : (g + 1) * D_FEAT],
                    in_=oimg[:, lo * D_FEAT : (g + 1) * D_FEAT],
                )
        assert t == nt

    nc.compile()
    return nc


_CACHE = {}


def _run(embeddings, src, dst, trace=False, trace_kwargs=None):
    img, dimg, rt, tpb, ngroups, unpack = _prep(embeddings, src, dst)
    key = (tpb, ngroups)
    if key not in _CACHE:
        _CACHE[key] = _build(tpb, ngroups)
    nc = _CACHE[key]

    in_maps = [
        {"tab": img[c], "dstv": dimg[c], "recip": rt[c]} for c in range(N_CORES)
    ]
    res = run_bass_kernel_spmd(
        nc,
        in_maps,
        core_ids=list(range(N_CORES)),
        trace=trace,
        **(trace_kwargs or {}),
    )
    outs = []
    for c in range(N_CORES):
        a = np.asarray(res.results[c]["out"])
        if a.dtype != np.float32:
            a = (a.view(np.uint16).astype(np.uint32) << 16).view(np.float32)
        outs.append(a.reshape(128, ngroups, D_FEAT))
    oimgs = np.stack(outs)  # [C, 128, G, 64]
    out = np.zeros((N_NODES, D_FEAT), np.float32)
    for nn, cc_, pp_, gg_ in unpack:
        out[nn] += oimgs[cc_, pp_, gg_, :]
    return out, res


def kernel(embeddings, src, dst):
    out, _ = _run(embeddings, src, dst, trace=False)
    return out


# revision 22
# speedup vs baseline: 1.0669x; 1.0669x over previous
"""GCN mean-aggregation (DGL copy_src -> mean by dst) on 8 NeuronCores.

Strategy (node-sharded, no collectives, host-packed edge records):
  - Host: nodes are assigned to cores by degree-balanced snake, then
    bin-packed per core into buckets of <=32 nodes and <=256 edges
    (degree-paired two-pointer fill, ~96% tile occupancy).  Each edge
    contributes a 128-byte feature record (64 x bf16 of its src row) and
    a 2-byte dst-slot value, laid out as SBUF images
    [128 partitions = edge%128, nt*128B] / [128, nt*2B], so the device
    streams everything with large sequential HWDGE DMAs (no per-edge
    gather descriptors, no SWDGE).
  - Device (identical program per core):
      * load the dst-slot table once; DVE builds fp8 one-hot tiles
        [128, 32] via batched is_equal(iota, slot) 16 tiles at a time
      * stream the feature table in chunks (quad buffered)
      * per 128-edge tile: one matmul psum[32q:32q+32, :64] +=
        onehot_fp8^T @ feat_bf16 (PE quadrant q = bucket%4, fp8 weights
        + bf16 moving; accumulate over the bucket's tiles)
      * per 4-bucket group: ACT copies psum -> bf16 out image scaled by
        recip = 1/max(deg,1) (host-precomputed, per-partition scalar)
      * 8 segment DMAs write the [128, ngroups*64] bf16 output image
  - Host: un-image per-node rows into the [100000, 64] f32 output.
"""

import bisect
import sys
from contextlib import ExitStack

import numpy as np

sys.path.insert(0, "/opt/trn_rl_repo")

import concourse.bass as bass  # noqa: E402
import concourse.mybir as mybir  # noqa: E402
import concourse.tile as tile  # noqa: E402
from concourse import bacc  # noqa: E402
from concourse.bass_utils import run_bass_kernel_spmd  # noqa: E402

N_NODES = 100000
N_EDGES = 1000000
D_FEAT = 64
N_CORES = 8
SLOTS = 32  # nodes per bucket (one-hot width)
EDGE_CAP = 256  # max edges per bucket
REC = 128  # bytes/record: 64 x bf16 feats
CHUNK = 32  # record tiles per DMA chunk
EDGE_CHUNK = 16  # smaller chunks at stream head/tail
OH_BATCH = 16  # one-hot tiles built per DVE instruction
OUT_SEGS = 8


def _f32_to_bf16_u16(x):
    u = np.ascontiguousarray(x, dtype=np.float32).view(np.uint32)
    r = ((u >> 16) & 1) + 0x7FFF  # round to nearest even
    return ((u + r) >> 16).astype(np.uint16)


def _pack_core(nodes, deg):
    """Exact-fill two-pointer bin-pack with node splitting.

    nodes: deg-desc order.  Returns list of bins; each bin is
    (edge_count, [(node, edges_assigned), ...]) with edge_count == EDGE_CAP
    for all but possibly the last bin, and <= SLOTS members per bin.
    Zero-degree nodes get no slot.
    """
    nodes = [n for n in nodes if deg[n] > 0]
    rem = {int(n): int(deg[n]) for n in nodes}
    bins = []
    i, j = 0, len(nodes) - 1
    cur, s = [], 0

    def close():
        nonlocal cur, s
        if cur:
            bins.append((s, cur))
            cur, s = [], 0

    while i <= j:
        # place the biggest remaining node, splitting across bins as needed
        n0 = int(nodes[i])
        i += 1
        r = rem[n0]
        while r > 0:
            space = EDGE_CAP - s
            if space == 0 or len(cur) >= SLOTS:
                close()
                space = EDGE_CAP
            take = min(r, space)
            cur.append((n0, take))
            s += take
            r -= take
        # fill with small nodes from the tail
        while j >= i and len(cur) < SLOTS and s < EDGE_CAP:
            nj = int(nodes[j])
            d = rem[nj]
            if s + d <= EDGE_CAP:
                cur.append((nj, d))
                s += d
                j -= 1
            else:
                take = EDGE_CAP - s
                cur.append((nj, take))
                rem[nj] = d - take
                s = EDGE_CAP
        if s >= EDGE_CAP or len(cur) >= SLOTS:
            close()
    close()
    return bins


def _prep(embeddings, src, dst):
    src = np.asarray(src, dtype=np.int64)
    dst = np.asarray(dst, dtype=np.int64)

    deg = np.bincount(dst, minlength=N_NODES)
    order = np.argsort(-deg, kind="stable")
    pos = np.arange(N_NODES) % (2 * N_CORES)
    core_pat = np.where(pos < N_CORES, pos, 2 * N_CORES - 1 - pos)
    core_of = np.empty(N_NODES, np.int64)
    core_of[order] = core_pat

    bins_c = []
    for c in range(N_CORES):
        nodes_c = order[core_of[order] == c]
        b = _pack_core(nodes_c, deg)
        b.sort(key=lambda t: -t[0])
        bins_c.append(b)
    nbmax = max(len(b) for b in bins_c)
    ngroups = -(-nbmax // 4)
    nb = ngroups * 4

    tpb = np.zeros(nb, np.int64)
    for b in bins_c:
        for r, (s, _) in enumerate(b):
            tpb[r] = max(tpb[r], -(-s // 128))
    tpb = np.maximum(tpb, 1)
    tile0 = np.zeros(nb + 1, np.int64)
    np.cumsum(tpb, out=tile0[1:])
    nt = int(tile0[-1])

    # per-node slot segments (a split node occupies slots in several buckets;
    # its edges fill them in segment order and the host sums the partial rows)
    recip = (1.0 / np.maximum(deg, 1)).astype(np.float32)
    rt = np.ones((N_CORES, 128, ngroups), np.float32)
    seg_of = [[] for _ in range(N_NODES)]  # node -> [(core, rank, slot, cnt)]
    for c in range(N_CORES):
        for r, (_, members) in enumerate(bins_c[c]):
            g, q = r // 4, r % 4
            for s, (n, cnt) in enumerate(members):
                seg_of[n].append((c, r, s, cnt))
                rt[c, q * 32 + s, g] = recip[n]

    # per-edge segment assignment: edges grouped by dst; the i-th edge of a
    # node goes to the segment covering index i
    eorder0 = np.argsort(dst, kind="stable")
    dcnt = np.bincount(dst, minlength=N_NODES)
    dstart = np.zeros(N_NODES + 1, np.int64)
    np.cumsum(dcnt, out=dstart[1:])
    r_in_node = np.arange(dst.shape[0], dtype=np.int64) - dstart[dst[eorder0]]

    ec = np.empty(dst.shape[0], np.int64)
    erank = np.empty(dst.shape[0], np.int64)
    eslot = np.empty(dst.shape[0], np.int64)
    # fast path: nodes with a single segment
    nseg = np.array([len(s) for s in seg_of], np.int64)
    s0 = np.zeros((N_NODES, 3), np.int64)
    for n in range(N_NODES):
        if seg_of[n]:
            s0[n] = seg_of[n][0][:3]
    dn = dst[eorder0]
    single = nseg[dn] == 1
    ec[single] = s0[dn[single], 0]
    erank[single] = s0[dn[single], 1]
    eslot[single] = s0[dn[single], 2]
    multi_nodes = np.nonzero(nseg > 1)[0]
    for n in multi_nodes:
        lo = dstart[n]
        hi = dstart[n + 1]
        idx = eorder0[lo:hi]  # this node's edges in placement order
        off = 0
        for (c, r, s, cnt) in seg_of[n]:
            sel = slice(lo, hi)
            pick = np.arange(lo, hi)[(r_in_node[lo:hi] >= off) & (r_in_node[lo:hi] < off + cnt)]
            ec[pick] = c
            erank[pick] = r
            eslot[pick] = s
            off += cnt

    # order edges by (core, rank) for bucket-sequential slots
    key = ec * nb + erank
    eorder1 = np.argsort(key, kind="stable")
    ks = key[eorder1]
    kcnt = np.bincount(ks, minlength=N_CORES * nb)
    kstart = np.zeros(kcnt.shape[0] + 1, np.int64)
    np.cumsum(kcnt, out=kstart[1:])
    k_in_bucket = np.arange(ks.shape[0], dtype=np.int64) - kstart[ks]

    et = tile0[erank[eorder1]] + (k_in_bucket >> 7)
    ep = k_in_bucket & 127
    ecc = ec[eorder1]
    esl = eslot[eorder1]
    esrc = src[eorder0][eorder1]

    featb = _f32_to_bf16_u16(embeddings)  # [N, 64] uint16

    FEAT = np.zeros((N_CORES * nt * 128, D_FEAT), np.uint16)
    DSTV = np.full((N_CORES * nt * 128,), SLOTS, np.uint8)
    rows = (ecc * nt + et) * 128 + ep
    FEAT[rows, :] = featb[esrc]
    DSTV[rows] = esl.astype(np.uint8)

    img = np.ascontiguousarray(
        FEAT.reshape(N_CORES, nt, 128, D_FEAT)
        .transpose(0, 2, 1, 3)
        .reshape(N_CORES, 128, nt * D_FEAT)
    ).view(np.int8)  # [C, 128, nt*128B]
    dimg = np.ascontiguousarray(
        DSTV.reshape(N_CORES, nt, 128).transpose(0, 2, 1)
    ).view(np.int8)  # [C, 128, nt] int8

    # unpack plan: per segment-position, (nodes, core, partition, group)
    maxseg = int(nseg.max()) if nseg.size else 0
    unpack = []
    for si in range(maxseg):
        nn = np.nonzero(nseg > si)[0]
        cc_ = np.array([seg_of[n][si][0] for n in nn], np.int64)
        rr_ = np.array([seg_of[n][si][1] for n in nn], np.int64)
        ss_ = np.array([seg_of[n][si][2] for n in nn], np.int64)
        unpack.append((nn, cc_, (rr_ % 4) * 32 + ss_, rr_ // 4))

    return img, dimg, rt, tuple(int(t) for t in tpb), ngroups, unpack


def _build(tpb, ngroups):
    f32 = mybir.dt.float32
    i8 = mybir.dt.int8
    bf16 = mybir.dt.bfloat16
    fp8 = mybir.dt.float8e4
    i32 = mybir.dt.int32
    nb = len(tpb)
    nt = sum(tpb)

    nc = bacc.Bacc("TRN2", target_bir_lowering=False, debug=False)
    tab = nc.dram_tensor("tab", [128, nt * REC], i8, kind="ExternalInput")
    dstv = nc.dram_tensor("dstv", [128, nt], i8, kind="ExternalInput")
    recip = nc.dram_tensor("recip", [128, ngroups], f32, kind="ExternalInput")
    out = nc.dram_tensor(
        "out", [128, ngroups * D_FEAT], bf16, kind="ExternalOutput"
    )

    bounds = [0]
    while bounds[-1] < nt:
        left = nt - bounds[-1]
        sz = (
            EDGE_CHUNK
            if (len(bounds) <= 2 or left <= 2 * EDGE_CHUNK + CHUNK)
            else CHUNK
        )
        bounds.append(min(bounds[-1] + sz, nt))

    with tile.TileContext(nc) as tc, ExitStack() as ctx:
        const_p = ctx.enter_context(tc.tile_pool(name="const", bufs=1))
        tab_p = ctx.enter_context(tc.tile_pool(name="tab", bufs=6))
        oh_p = ctx.enter_context(tc.tile_pool(name="oh", bufs=8))
        ps_p = ctx.enter_context(tc.tile_pool(name="ps", bufs=8, space="PSUM"))
        out_p = ctx.enter_context(tc.tile_pool(name="outp", bufs=1))

        dv = const_p.tile([128, nt], i8)
        rc = const_p.tile([128, ngroups], f32)

        iota_i = const_p.tile([128, SLOTS], i32)
        nc.gpsimd.iota(iota_i[:], pattern=[[1, SLOTS]], base=0, channel_multiplier=0)
        iota_b = const_p.tile([128, SLOTS], i8)
        nc.vector.tensor_copy(out=iota_b[:], in_=iota_i[:])

        oimg = out_p.tile([128, ngroups * D_FEAT], bf16)

        chunks = []

        def chunk_for(t):
            c = bisect.bisect_right(bounds, t) - 1
            while len(chunks) <= c:
                cc = len(chunks)
                t0b, t1b = bounds[cc], bounds[cc + 1]
                ctile = tab_p.tile([128, (t1b - t0b) * REC], i8, tag="chunk")
                nc.sync.dma_start(
                    out=ctile[:], in_=tab[:, t0b * REC : t1b * REC]
                )
                chunks.append(ctile)
            return chunks[c], (t - bounds[c]) * REC

        ohtiles = []

        def oh_for(t):
            c = t // OH_BATCH
            while len(ohtiles) <= c:
                cc = len(ohtiles)
                k = min(OH_BATCH, nt - cc * OH_BATCH)
                oht = oh_p.tile([128, k * SLOTS], fp8, tag="oh")
                nc.vector.tensor_tensor(
                    out=oht[:].rearrange("p (b f) -> p b f", b=k),
                    in0=iota_b[:, None, :].broadcast_to([128, k, SLOTS]),
                    in1=dv[:, cc * OH_BATCH : cc * OH_BATCH + k][
                        :, :, None
                    ].broadcast_to([128, k, SLOTS]),
                    op=mybir.AluOpType.is_equal,
                )
                ohtiles.append(oht)
            return ohtiles[c], (t - c * OH_BATCH) * SLOTS

        seg_end = [((s + 1) * ngroups) // OUT_SEGS for s in range(OUT_SEGS)]

        chunk_for(0)  # start streaming the table before the small const loads
        if bounds[1] < nt:
            chunk_for(bounds[1])
        nc.sync.dma_start(out=dv[:], in_=dstv[:, :])
        nc.sync.dma_start(out=rc[:], in_=recip[:, :])

        t = 0
        for g in range(ngroups):
            psum = ps_p.tile([128, D_FEAT], f32)
            for q in range(4):
                r = g * 4 + q
                for j in range(tpb[r]):
                    ctile, o = chunk_for(t)
                    oht, oo = oh_for(t)
                    nc.tensor.matmul(
                        out=psum[32 * q : 32 * (q + 1), :],
                        lhsT=oht[:, oo : oo + SLOTS],
                        rhs=ctile[:, o : o + REC].bitcast(bf16),
                        start=(j == 0),
                        stop=(j == tpb[r] - 1),
                        tile_position=(0, 32 * q),
                    )
                    t += 1
            nc.scalar.activation(
                out=oimg[:, g * D_FEAT : (g + 1) * D_FEAT],
                in_=psum[:],
                func=mybir.ActivationFunctionType.Copy,
                scale=rc[:, g : g + 1],
            )
            if g + 1 in seg_end:
                s0 = seg_end.index(g + 1)
                lo = 0 if s0 == 0 else seg_end[s0 - 1]
                nc.sync.dma_start(
                    out=out[:, lo * D_FEAT : (g + 1) * D_FEAT],
                    in_=oimg[:, lo * D_FEAT : (g + 1) * D_FEAT],
                )
        assert t == nt

    nc.compile()
    return nc


_CACHE = {}


def _run(embeddings, src, dst, trace=False, trace_kwargs=None):
    img, dimg, rt, tpb, ngroups, unpack = _prep(embeddings, src, dst)
    key = (tpb, ngroups)
    if key not in _CACHE:
        _CACHE[key] = _build(tpb, ngroups)
    nc = _CACHE[key]

    in_maps = [
        {"tab": img[c], "dstv": dimg[c], "recip": rt[c]} for c in range(N_CORES)
    ]
    res = run_bass_kernel_spmd(
        nc,
        in_maps,
        core_ids=list(range(N_CORES)),
        trace=trace,
        **(trace_kwargs or {}),
    )
    outs = []
    for c in range(N_CORES):
        a = np.asarray(res.results[c]["out"])
        if a.dtype != np.float32:
            a = (a.view(np.uint16).astype(np.uint32) << 16).view(np.float32)
        outs.append(a.reshape(128, ngroups, D_FEAT))
    oimgs = np.stack(outs)  # [C, 128, G, 64]
    out = np.zeros((N_NODES, D_FEAT), np.float32)
    for nn, cc_, pp_, gg_ in unpack:
        out[nn] += oimgs[cc_, pp_, gg_, :]
    return out, res


def kernel(embeddings, src, dst):
    out, _ = _run(embeddings, src, dst, trace=False)
    return out
